# revision 24
# baseline (speedup 1.0000x reference)
"""NCutLoss3D Trainium2 kernel.

Math (per class k, batch b; p = labels[b,k], x = inputs[b,0], all 128^3):
  m   = sum(x*p) / (sum(p) + V*1e-5)
  w   = exp(-((x-m)^2)^2)
  r   = blur3d(p)            (separable 9-tap peak-1 gaussian, radius 4, zero pad)
  num = <blur3d(p*w), p> == <r, p*w>   (blur is self-adjoint)
  den = <blur3d(w),   p> == <r, w>
  loss += |num/(den+1e-6)|; return 4 - loss

Blur as banded-Toeplitz matmul G[i,j]=exp(-(i-j)^2/50), |i-j|<=4.
Data-stationary matmul out = lhsT.T @ G blurs the partition axis and swaps
partition<->M, giving a 3-pass layout cycle:
  p[d,(h,w)] -> A[h,(d,w)] -> B[w,(h,d)] -> r[d,(h,w)]  (natural layout)

Reductions avoid TensorTensorReduce (wedges DVE on this HW): products via
DVE tensor_mul, sums via ACT copy+accum_out and PE ones-matmuls that
accumulate into persistent PSUM banks across iterations.

Sharding: core c handles (b,k) = (c//4, c%4); zero cross-core traffic.
Each core returns out[1,2] = (num, den); host combines in float64.
"""
import sys

import numpy as np

_TRN_REPO = "/opt/trn_rl_repo"
if _TRN_REPO not in sys.path:
    sys.path.insert(0, _TRN_REPO)

RADIUS = 4
SIGMA1 = 5.0
V = 128 * 128 * 128
CH = 2048
NCH = 16384 // CH  # 8 stream chunks

_CACHE = {}
_TRACE = False
_LAST = {}
_DERF = True  # use Derivative_Erf for exp(-d1^2); CoreSim lacks it


def _gmat() -> np.ndarray:
    g = np.exp(-(np.arange(-RADIUS, RADIUS + 1, dtype=np.float64) ** 2)
               / (2.0 * SIGMA1 ** 2))
    G = np.zeros((128, 128), np.float64)
    for i in range(128):
        for j in range(max(0, i - RADIUS), min(128, i + RADIUS + 1)):
            G[i, j] = g[j - i + RADIUS]
    return G.astype(np.float32)


def _build_nc(stage: int = 5):
    import concourse.bass as bass
    import concourse.tile as tile
    from concourse import mybir

    f32 = mybir.dt.float32
    bf16 = mybir.dt.bfloat16
    AF = mybir.ActivationFunctionType
    PSUM = bass.MemorySpace.PSUM

    nc = bass.Bass(target_bir_lowering=False)
    p_d = nc.dram_tensor("p", (128, 16384), f32, kind="ExternalInput")
    x_d = nc.dram_tensor("x", (128, 16384), f32, kind="ExternalInput")
    g_d = nc.dram_tensor("g", (128, 128), f32, kind="ExternalInput")
    out_d = nc.dram_tensor("out", (1, 2), f32, kind="ExternalOutput")

    with tile.TileContext(nc) as tc:
        frees = []  # keep tc.tile free-closures alive until explicit release

        def T(shape, dtype, name, space=bass.MemorySpace.SBUF):
            t, f = tc.tile(shape, dtype, name=name, space=space)
            frees.append(f)
            return t

        p16 = T((128, 16384), bf16, "p16")
        A16 = T((128, 16384), bf16, "A16")
        B16 = T((128, 16384), bf16, "B16")
        W16 = T((128, 16384), bf16, "W16")
        gf = T((128, 128), f32, "gf")
        g16 = T((128, 128), bf16, "g16")
        sp_parts = T((128, NCH), f32, "sp_parts")
        ones_col = T((128, 1), f32, "ones_col")
        ones16 = T((128, 1), bf16, "ones16")
        negones_row = T((1, 128), f32, "negones_row")
        negm_sb = T((128, 1), f32, "negm_sb")
        sp_col = T((128, 1), f32, "sp_col")
        spb = T((1, 1), f32, "spb")
        rcp = T((1, 1), f32, "rcp")
        sxp_sb = T((1, 1), f32, "sxp_sb")
        m_sb = T((1, 1), f32, "m_sb")
        out_sb = T((1, 2), f32, "out_sb")
        xpacc = T((1, 512), f32, "xpacc", space=PSUM)
        q1acc = T((1, 512), f32, "q1acc", space=PSUM)
        q2acc = T((1, 512), f32, "q2acc", space=PSUM)
        psscr = T((128, 1), f32, "psscr", space=PSUM)

        with tc.tile_pool(name="io", bufs=2) as io_pool, \
             tc.tile_pool(name="sc", bufs=2) as sc_pool, \
             tc.tile_pool(name="mm", bufs=2, space="PSUM") as mm_pool:

            nc.vector.memset(ones_col, 1.0)
            nc.vector.memset(negones_row, -1.0)
            nc.scalar.copy(out=ones16, in_=ones_col)
            nc.sync.dma_start(out=gf, in_=g_d[:, :])
            nc.scalar.copy(out=g16, in_=gf)

            # ---- stage A part 1: stream ALL of p first; cast p->bf16 + sp
            # sums. p-first lets pass 1 (which needs full p16) start while x
            # is still streaming.
            for i in range(NCH):
                sl = slice(i * CH, (i + 1) * CH)
                pf = io_pool.tile((128, CH), f32)
                nc.sync.dma_start(out=pf, in_=p_d[:, sl])
                nc.scalar.activation(out=p16[:, sl], in_=pf, func=AF.Copy,
                                     bias=0.0, scale=1.0,
                                     accum_out=sp_parts[:, i:i + 1])

            if stage >= 3:
                # ---- pass 1: blur d.  A[h, k*128+w] = sum_d G[d,k] p[d,h,w]
                # issued before the x-loop so PE/ACT overlap the x stream
                p16_r = p16.rearrange("p (h w) -> p h w", w=128)
                A16_r = A16.rearrange("p (k w) -> p k w", w=128)
                for t in range(16):
                    ps = mm_pool.tile((128, 1024), f32)
                    for j in range(8):
                        w0 = t * 8 + j
                        nc.tensor.matmul(out=ps[:, j * 128:(j + 1) * 128],
                                         lhsT=p16_r[:, :, w0], rhs=g16,
                                         start=True, stop=True)
                    nc.scalar.activation(
                        out=A16_r[:, :, t * 8:(t + 1) * 8],
                        in_=ps.rearrange("p (t k) -> p k t", k=128),
                        func=AF.Copy, bias=0.0, scale=1.0)

            # ---- stage A part 2: stream x; sxp via DVE mult + PE
            # ones-matmul accumulating into persistent PSUM bank xpacc
            for i in range(NCH):
                sl = slice(i * CH, (i + 1) * CH)
                xf = io_pool.tile((128, CH), f32)
                nc.sync.dma_start(out=xf, in_=x_d[:, sl])
                xpj = sc_pool.tile((128, CH), bf16)
                nc.vector.tensor_mul(out=xpj, in0=xf, in1=p16[:, sl])
                for j in range(CH // 512):
                    nc.tensor.matmul(out=xpacc,
                                     lhsT=ones16,
                                     rhs=xpj[:, j * 512:(j + 1) * 512],
                                     start=(i == 0 and j == 0),
                                     stop=(i == NCH - 1 and j == CH // 512 - 1))

            if stage == 1:
                nc.scalar.copy(out=out_sb, in_=sp_parts[0:1, 0:2])

            if stage >= 2:
                # ---- m chain: m = sxp / (sp + V*1e-5); broadcast -m
                j8 = sc_pool.tile((128, NCH), f32)
                nc.scalar.activation(out=j8, in_=sp_parts, func=AF.Copy,
                                     bias=0.0, scale=1.0, accum_out=sp_col)
                j512 = sc_pool.tile((1, 512), f32)
                nc.scalar.activation(out=j512, in_=xpacc, func=AF.Copy,
                                     bias=0.0, scale=1.0, accum_out=sxp_sb)
                nc.tensor.matmul(out=psscr[0:1, 0:1], lhsT=ones_col,
                                 rhs=sp_col, start=True, stop=True)
                nc.scalar.activation(out=spb, in_=psscr[0:1, 0:1],
                                     func=AF.Copy,
                                     bias=float(V * 1e-5), scale=1.0)
                nc.vector.reciprocal(out=rcp, in_=spb)
                nc.vector.tensor_mul(out=m_sb, in0=sxp_sb, in1=rcp)
                nc.tensor.matmul(out=psscr, lhsT=negones_row, rhs=m_sb,
                                 start=True, stop=True)
                nc.scalar.copy(out=negm_sb, in_=psscr)
            if stage == 2:
                nc.scalar.copy(out=out_sb[0:1, 0:1], in_=spb)
                nc.scalar.copy(out=out_sb[0:1, 1:2], in_=m_sb)

            if stage == 3:
                nc.scalar.activation(out=out_sb, in_=A16[0:1, 0:2],
                                     func=AF.Copy, bias=0.0, scale=1.0)

            if stage >= 4:
                # ---- pass 2: blur h.  B[w, k*128+d] = sum_h G[h,k] A[h,d,w]
                # overlapped: re-stream x, w-pipeline d1=(x-m)^2, w=exp(-d1^2)
                B16_r = B16.rearrange("p (k d) -> p k d", d=128)
                for t in range(16):
                    if t % 2 == 0:
                        i = t // 2
                        sl = slice(i * CH, (i + 1) * CH)
                        xf = io_pool.tile((128, CH), f32)
                        nc.sync.dma_start(out=xf, in_=x_d[:, sl])
                        d1 = sc_pool.tile((128, CH), bf16)
                        nc.scalar.activation(out=d1, in_=xf, func=AF.Square,
                                             bias=negm_sb, scale=1.0)
                        if _DERF:
                            # D_Erf(u) = (2/sqrt(pi)) exp(-u^2); host rescales
                            nc.scalar.activation(out=W16[:, sl], in_=d1,
                                                 func=AF.Derivative_Erf,
                                                 bias=0.0, scale=1.0)
                        else:
                            d2 = sc_pool.tile((128, CH), bf16)
                            nc.vector.tensor_mul(out=d2, in0=d1, in1=d1)
                            nc.scalar.activation(out=W16[:, sl], in_=d2,
                                                 func=AF.Exp, bias=0.0,
                                                 scale=-1.0)
                    ps = mm_pool.tile((128, 1024), f32)
                    for j in range(8):
                        d0 = t * 8 + j
                        nc.tensor.matmul(out=ps[:, j * 128:(j + 1) * 128],
                                         lhsT=A16[:, d0 * 128:(d0 + 1) * 128],
                                         rhs=g16, start=True, stop=True)
                    nc.vector.tensor_copy(
                        out=B16_r[:, :, t * 8:(t + 1) * 8],
                        in_=ps.rearrange("p (t k) -> p k t", k=128))
            if stage == 4:
                nc.scalar.activation(out=out_sb, in_=B16[0:1, 0:2],
                                     func=AF.Copy, bias=0.0, scale=1.0)

            if stage >= 5:
                # ---- pass 3: blur w.  r[d, h*128+k] = sum_w G[w,k] B[w,h,d]
                # s = r*w; den q1 = sum(s); num q2 = <s, p> via PE ones-matmul
                for t in range(16):
                    sl = slice(t * 1024, (t + 1) * 1024)
                    ps = mm_pool.tile((128, 1024), f32)
                    for j in range(8):
                        h0 = t * 8 + j
                        nc.tensor.matmul(out=ps[:, j * 128:(j + 1) * 128],
                                         lhsT=B16[:, h0 * 128:(h0 + 1) * 128],
                                         rhs=g16, start=True, stop=True)
                    s16 = sc_pool.tile((128, 1024), bf16)
                    nc.vector.tensor_mul(out=s16, in0=ps, in1=W16[:, sl])
                    sp16 = sc_pool.tile((128, 1024), bf16)
                    nc.vector.tensor_mul(out=sp16, in0=s16, in1=p16[:, sl])
                    for j in range(2):
                        qs = slice(j * 512, (j + 1) * 512)
                        nc.tensor.matmul(out=q1acc, lhsT=ones16,
                                         rhs=s16[:, qs],
                                         start=(t == 0 and j == 0),
                                         stop=(t == 15 and j == 1))
                        nc.tensor.matmul(out=q2acc, lhsT=ones16,
                                         rhs=sp16[:, qs],
                                         start=(t == 0 and j == 0),
                                         stop=(t == 15 and j == 1))

                # ---- final reduce: out[0,:] = (num, den)
                jq2 = sc_pool.tile((1, 512), f32)
                nc.scalar.activation(out=jq2, in_=q2acc, func=AF.Copy,
                                     bias=0.0, scale=1.0,
                                     accum_out=out_sb[0:1, 0:1])
                jq1 = sc_pool.tile((1, 512), f32)
                nc.scalar.activation(out=jq1, in_=q1acc, func=AF.Copy,
                                     bias=0.0, scale=1.0,
                                     accum_out=out_sb[0:1, 1:2])
            nc.sync.dma_start(out=out_d[:, :], in_=out_sb)

        for f in reversed(frees):
            f()

    import bass_rust.bass_rust as _br
    _br.generate_event_semaphores(nc)
    mybir.codegen_inst_isa_subclasses(nc)
    return nc


def kernel(labels: np.ndarray, inputs: np.ndarray) -> np.ndarray:
    from concourse.bass_utils import run_bass_kernel_spmd

    if "nc" not in _CACHE:
        _CACHE["nc"] = _build_nc()
    nc = _CACHE["nc"]

    labels = np.ascontiguousarray(np.asarray(labels, dtype=np.float32))
    x = np.ascontiguousarray(np.asarray(inputs, dtype=np.float32))
    G = _gmat()
    in_maps = []
    for c in range(8):
        b, k = c // 4, c % 4
        in_maps.append({
            "p": np.ascontiguousarray(labels[b, k].reshape(128, 16384)),
            "x": np.ascontiguousarray(x[b, 0].reshape(128, 16384)),
            "g": G,
        })
    res = run_bass_kernel_spmd(nc, in_maps, core_ids=list(range(8)),
                               trace=_TRACE)
    _LAST["res"] = res
    outs = [np.asarray(r["out"], dtype=np.float64) for r in res.results]
    if _DERF:
        scale = 0.5 * np.sqrt(np.pi)
        outs = [o * scale for o in outs]

    loss = 0.0
    for k in range(4):
        num = outs[k][0, 0] + outs[4 + k][0, 0]
        den = outs[k][0, 1] + outs[4 + k][0, 1]
        loss += abs(num / (den + 1e-6))
    return np.float32(4.0 - loss)


# revision 27
# speedup vs baseline: 34480.7529x; 34480.7529x over previous
"""NCutLoss3D Trainium2 kernel.

Math (per class k, batch b; p = labels[b,k], x = inputs[b,0], all 128^3):
  m   = sum(x*p) / (sum(p) + V*1e-5)
  w   = exp(-((x-m)^2)^2)
  r   = blur3d(p)            (separable 9-tap peak-1 gaussian, radius 4, zero pad)
  num = <blur3d(p*w), p> == <r, p*w>   (blur is self-adjoint)
  den = <blur3d(w),   p> == <r, w>
  loss += |num/(den+1e-6)|; return 4 - loss

Blur as banded-Toeplitz matmul G[i,j]=exp(-(i-j)^2/50), |i-j|<=4.
Data-stationary matmul out = lhsT.T @ G blurs the partition axis and swaps
partition<->M, giving a 3-pass layout cycle:
  p[d,(h,w)] -> A[h,(d,w)] -> B[w,(h,d)] -> r[d,(h,w)]  (natural layout)

Reductions avoid TensorTensorReduce (wedges DVE on this HW): products via
DVE tensor_mul, sums via ACT copy+accum_out and PE ones-matmuls that
accumulate into persistent PSUM banks across iterations.

Sharding: core c handles (b,k) = (c//4, c%4); zero cross-core traffic.
Each core returns out[1,2] = (num, den); host combines in float64.
"""
import sys

import numpy as np

_TRN_REPO = "/opt/trn_rl_repo"
if _TRN_REPO not in sys.path:
    sys.path.insert(0, _TRN_REPO)

RADIUS = 4
SIGMA1 = 5.0
V = 128 * 128 * 128
CH = 2048
NCH = 16384 // CH  # 8 stream chunks

_CACHE = {}
_TRACE = False
_LAST = {}
_DERF = True  # use Derivative_Erf for exp(-d1^2); CoreSim lacks it
_GPS = False  # Pool (gpsimd) sp16 modeled slower than DVE; keep on DVE


def _gmat() -> np.ndarray:
    g = np.exp(-(np.arange(-RADIUS, RADIUS + 1, dtype=np.float64) ** 2)
               / (2.0 * SIGMA1 ** 2))
    G = np.zeros((128, 128), np.float64)
    for i in range(128):
        for j in range(max(0, i - RADIUS), min(128, i + RADIUS + 1)):
            G[i, j] = g[j - i + RADIUS]
    return G.astype(np.float32)


def _build_nc(stage: int = 5):
    import concourse.bass as bass
    import concourse.tile as tile
    from concourse import mybir

    f32 = mybir.dt.float32
    bf16 = mybir.dt.bfloat16
    AF = mybir.ActivationFunctionType
    PSUM = bass.MemorySpace.PSUM

    nc = bass.Bass(target_bir_lowering=False)
    p_d = nc.dram_tensor("p", (128, 16384), f32, kind="ExternalInput")
    x_d = nc.dram_tensor("x", (128, 16384), f32, kind="ExternalInput")
    g_d = nc.dram_tensor("g", (128, 128), f32, kind="ExternalInput")
    out_d = nc.dram_tensor("out", (1, 2), f32, kind="ExternalOutput")

    with tile.TileContext(nc) as tc:
        frees = []  # keep tc.tile free-closures alive until explicit release

        def T(shape, dtype, name, space=bass.MemorySpace.SBUF):
            t, f = tc.tile(shape, dtype, name=name, space=space)
            frees.append(f)
            return t

        p16 = T((128, 16384), bf16, "p16")
        A16 = T((128, 16384), bf16, "A16")
        B16 = T((128, 16384), bf16, "B16")
        W16 = T((128, 16384), bf16, "W16")
        gf = T((128, 128), f32, "gf")
        g16 = T((128, 128), bf16, "g16")
        sp_parts = T((128, NCH), f32, "sp_parts")
        ones_col = T((128, 1), f32, "ones_col")
        ones16 = T((128, 1), bf16, "ones16")
        negones_row = T((1, 128), f32, "negones_row")
        negm_sb = T((128, 1), f32, "negm_sb")
        sp_col = T((128, 1), f32, "sp_col")
        spb = T((1, 1), f32, "spb")
        rcp = T((1, 1), f32, "rcp")
        sxp_sb = T((1, 1), f32, "sxp_sb")
        m_sb = T((1, 1), f32, "m_sb")
        out_sb = T((1, 2), f32, "out_sb")
        xpacc = T((1, 512), f32, "xpacc", space=PSUM)
        q1acc = T((1, 512), f32, "q1acc", space=PSUM)
        q2acc = T((1, 512), f32, "q2acc", space=PSUM)
        psscr = T((128, 1), f32, "psscr", space=PSUM)

        with tc.tile_pool(name="io", bufs=2) as io_pool, \
             tc.tile_pool(name="sc", bufs=2) as sc_pool, \
             tc.tile_pool(name="mm", bufs=2, space="PSUM") as mm_pool:

            nc.vector.memset(ones_col, 1.0)
            nc.vector.memset(negones_row, -1.0)
            nc.scalar.copy(out=ones16, in_=ones_col)
            nc.sync.dma_start(out=gf, in_=g_d[:, :])
            nc.scalar.copy(out=g16, in_=gf)

            # ---- stage A part 1: stream ALL of p first; cast p->bf16 + sp
            # sums. p-first lets pass 1 (which needs full p16) start while x
            # is still streaming.
            for i in range(NCH):
                sl = slice(i * CH, (i + 1) * CH)
                pf = io_pool.tile((128, CH), f32)
                nc.sync.dma_start(out=pf, in_=p_d[:, sl])
                nc.scalar.activation(out=p16[:, sl], in_=pf, func=AF.Copy,
                                     bias=0.0, scale=1.0,
                                     accum_out=sp_parts[:, i:i + 1])

            if stage >= 3:
                # ---- pass 1: blur d.  A[h, k*128+w] = sum_d G[d,k] p[d,h,w]
                # issued before the x-loop so PE/ACT overlap the x stream
                p16_r = p16.rearrange("p (h w) -> p h w", w=128)
                A16_r = A16.rearrange("p (k w) -> p k w", w=128)
                for t in range(16):
                    ps = mm_pool.tile((128, 1024), f32)
                    for j in range(8):
                        w0 = t * 8 + j
                        nc.tensor.matmul(out=ps[:, j * 128:(j + 1) * 128],
                                         lhsT=p16_r[:, :, w0], rhs=g16,
                                         start=True, stop=True)
                    nc.scalar.activation(
                        out=A16_r[:, :, t * 8:(t + 1) * 8],
                        in_=ps.rearrange("p (t k) -> p k t", k=128),
                        func=AF.Copy, bias=0.0, scale=1.0)

            # ---- stage A part 2: stream x; sxp via DVE mult + PE
            # ones-matmul accumulating into persistent PSUM bank xpacc
            for i in range(NCH):
                sl = slice(i * CH, (i + 1) * CH)
                xf = io_pool.tile((128, CH), f32)
                nc.sync.dma_start(out=xf, in_=x_d[:, sl])
                xpj = sc_pool.tile((128, CH), bf16)
                nc.vector.tensor_mul(out=xpj, in0=xf, in1=p16[:, sl])
                for j in range(CH // 512):
                    nc.tensor.matmul(out=xpacc,
                                     lhsT=ones16,
                                     rhs=xpj[:, j * 512:(j + 1) * 512],
                                     start=(i == 0 and j == 0),
                                     stop=(i == NCH - 1 and j == CH // 512 - 1))

            if stage == 1:
                nc.scalar.copy(out=out_sb, in_=sp_parts[0:1, 0:2])

            if stage >= 2:
                # ---- m chain: m = sxp / (sp + V*1e-5); broadcast -m
                j8 = sc_pool.tile((128, NCH), f32)
                nc.scalar.activation(out=j8, in_=sp_parts, func=AF.Copy,
                                     bias=0.0, scale=1.0, accum_out=sp_col)
                j512 = sc_pool.tile((1, 512), f32)
                nc.scalar.activation(out=j512, in_=xpacc, func=AF.Copy,
                                     bias=0.0, scale=1.0, accum_out=sxp_sb)
                nc.tensor.matmul(out=psscr[0:1, 0:1], lhsT=ones_col,
                                 rhs=sp_col, start=True, stop=True)
                nc.scalar.activation(out=spb, in_=psscr[0:1, 0:1],
                                     func=AF.Copy,
                                     bias=float(V * 1e-5), scale=1.0)
                nc.vector.reciprocal(out=rcp, in_=spb)
                nc.vector.tensor_mul(out=m_sb, in0=sxp_sb, in1=rcp)
                nc.tensor.matmul(out=psscr, lhsT=negones_row, rhs=m_sb,
                                 start=True, stop=True)
                nc.scalar.copy(out=negm_sb, in_=psscr)
            if stage == 2:
                nc.scalar.copy(out=out_sb[0:1, 0:1], in_=spb)
                nc.scalar.copy(out=out_sb[0:1, 1:2], in_=m_sb)

            if stage == 3:
                nc.scalar.activation(out=out_sb, in_=A16[0:1, 0:2],
                                     func=AF.Copy, bias=0.0, scale=1.0)

            if stage >= 4:
                # ---- pass 2: blur h.  B[w, k*128+d] = sum_h G[h,k] A[h,d,w]
                # overlapped: re-stream x, w-pipeline d1=(x-m)^2, w=exp(-d1^2)
                B16_r = B16.rearrange("p (k d) -> p k d", d=128)
                for t in range(16):
                    if t % 2 == 0:
                        i = t // 2
                        sl = slice(i * CH, (i + 1) * CH)
                        xf = io_pool.tile((128, CH), f32)
                        nc.sync.dma_start(out=xf, in_=x_d[:, sl])
                        d1 = sc_pool.tile((128, CH), bf16)
                        nc.scalar.activation(out=d1, in_=xf, func=AF.Square,
                                             bias=negm_sb, scale=1.0)
                        if _DERF:
                            # D_Erf(u) = (2/sqrt(pi)) exp(-u^2); host rescales
                            nc.scalar.activation(out=W16[:, sl], in_=d1,
                                                 func=AF.Derivative_Erf,
                                                 bias=0.0, scale=1.0)
                        else:
                            d2 = sc_pool.tile((128, CH), bf16)
                            nc.vector.tensor_mul(out=d2, in0=d1, in1=d1)
                            nc.scalar.activation(out=W16[:, sl], in_=d2,
                                                 func=AF.Exp, bias=0.0,
                                                 scale=-1.0)
                    ps = mm_pool.tile((128, 1024), f32)
                    for j in range(8):
                        d0 = t * 8 + j
                        nc.tensor.matmul(out=ps[:, j * 128:(j + 1) * 128],
                                         lhsT=A16[:, d0 * 128:(d0 + 1) * 128],
                                         rhs=g16, start=True, stop=True)
                    nc.vector.tensor_copy(
                        out=B16_r[:, :, t * 8:(t + 1) * 8],
                        in_=ps.rearrange("p (t k) -> p k t", k=128))
            if stage == 4:
                nc.scalar.activation(out=out_sb, in_=B16[0:1, 0:2],
                                     func=AF.Copy, bias=0.0, scale=1.0)

            if stage >= 5:
                # ---- pass 3: blur w.  r[d, h*128+k] = sum_w G[w,k] B[w,h,d]
                # s = r*w; den q1 = sum(s); num q2 = <s, p> via PE ones-matmul
                for t in range(16):
                    sl = slice(t * 1024, (t + 1) * 1024)
                    ps = mm_pool.tile((128, 1024), f32)
                    for j in range(8):
                        h0 = t * 8 + j
                        nc.tensor.matmul(out=ps[:, j * 128:(j + 1) * 128],
                                         lhsT=B16[:, h0 * 128:(h0 + 1) * 128],
                                         rhs=g16, start=True, stop=True)
                    s16 = sc_pool.tile((128, 1024), bf16)
                    nc.vector.tensor_mul(out=s16, in0=ps, in1=W16[:, sl])
                    sp16 = sc_pool.tile((128, 1024), bf16)
                    if _GPS:
                        nc.gpsimd.tensor_mul(out=sp16, in0=s16,
                                             in1=p16[:, sl])
                    else:
                        nc.vector.tensor_mul(out=sp16, in0=s16,
                                             in1=p16[:, sl])
                    for j in range(2):
                        qs = slice(j * 512, (j + 1) * 512)
                        nc.tensor.matmul(out=q1acc, lhsT=ones16,
                                         rhs=s16[:, qs],
                                         start=(t == 0 and j == 0),
                                         stop=(t == 15 and j == 1))
                        nc.tensor.matmul(out=q2acc, lhsT=ones16,
                                         rhs=sp16[:, qs],
                                         start=(t == 0 and j == 0),
                                         stop=(t == 15 and j == 1))

                # ---- final reduce: out[0,:] = (num, den)
                jq2 = sc_pool.tile((1, 512), f32)
                nc.scalar.activation(out=jq2, in_=q2acc, func=AF.Copy,
                                     bias=0.0, scale=1.0,
                                     accum_out=out_sb[0:1, 0:1])
                jq1 = sc_pool.tile((1, 512), f32)
                nc.scalar.activation(out=jq1, in_=q1acc, func=AF.Copy,
                                     bias=0.0, scale=1.0,
                                     accum_out=out_sb[0:1, 1:2])
            nc.sync.dma_start(out=out_d[:, :], in_=out_sb)

        for f in reversed(frees):
            f()

    import bass_rust.bass_rust as _br
    _br.generate_event_semaphores(nc)
    mybir.codegen_inst_isa_subclasses(nc)
    return nc


def kernel(labels: np.ndarray, inputs: np.ndarray) -> np.ndarray:
    from concourse.bass_utils import run_bass_kernel_spmd

    if "nc" not in _CACHE:
        _CACHE["nc"] = _build_nc()
    nc = _CACHE["nc"]

    labels = np.ascontiguousarray(np.asarray(labels, dtype=np.float32))
    x = np.ascontiguousarray(np.asarray(inputs, dtype=np.float32))
    G = _gmat()
    in_maps = []
    for c in range(8):
        b, k = c // 4, c % 4
        in_maps.append({
            "p": np.ascontiguousarray(labels[b, k].reshape(128, 16384)),
            "x": np.ascontiguousarray(x[b, 0].reshape(128, 16384)),
            "g": G,
        })
    res = run_bass_kernel_spmd(nc, in_maps, core_ids=list(range(8)),
                               trace=_TRACE)
    _LAST["res"] = res
    outs = [np.asarray(r["out"], dtype=np.float64) for r in res.results]
    if _DERF:
        scale = 0.5 * np.sqrt(np.pi)
        outs = [o * scale for o in outs]

    loss = 0.0
    for k in range(4):
        num = outs[k][0, 0] + outs[4 + k][0, 0]
        den = outs[k][0, 1] + outs[4 + k][0, 1]
        loss += abs(num / (den + 1e-6))
    return np.float32(4.0 - loss)
